# revision 12
# baseline (speedup 1.0000x reference)
"""Causal self-attention Bass/TRN2 kernel for nn_CausalSelfAttention.

Shapes (hardcoded): query [2, 2048, 1024], 16 heads, d=64.
Sharding: 8 cores = 2 batches x 4 head-groups (4 heads per core, tensor
parallel on QKV/proj weight columns). Each core computes a partial output
projection out_t = Wp_slice^T @ y^T (shape [1024, 2048] bf16); host sums the
4 partials per batch in f32, transposes, and adds bp.

Host-side prep (free): X transposed to X^T and cast to bf16; weights cast to
bf16. All device compute in bf16 (inputs) with f32 PSUM accumulation.

Per-core pipeline, streamed over 4 query chunks g of 512:
  1. Q^T, K^T chunk = Wq/Wk_slice^T @ X^T chunk (8 k-steps, f32 PSUM,
     ACT Identity copy -> bf16 SBUF, + bias when present).
  2. V tiles = X^T chunk^T-contraction @ Wv_slice (natural [t, d] layout),
     Pool copy into va[s, h, tile, 0:64]; va[..., 64] = 1 (denominator ones).
  3. Attention per head-pair hp: S^T_j = k^T_j ^T @ q^T (2 heads packed in a
     [128,1024] PSUM tile); ACT exp (scale=1/8, no max-subtraction -- scores
     are bounded for this problem) -> bf16 P; causal diag-block zeroing via
     Pool affine_select on P; PV + denominator (ones column of va) into a
     [128,1024] PSUM tile (rows 0:64 y^T, row 64 den).
  4. Normalize: DVE reciprocal of den row, PE K=1 ones-matmul broadcast to 64
     rows, Pool copy PSUM->SBUF, DVE multiply -> yt bf16 (odd head moved to
     partitions 64:128 via SBUF-SBUF DMA).
  5. outproj(g-1) emitted between hp0 and hp1 of chunk g so PE never waits
     on the normalization chain: out_t = Wp_slice^T @ y^T, DVE copy -> bf16,
     DMA out.

This walrus build accepts only ONE sync-wait command per TPB instruction, so
after Tile scheduling we hoist excess waits into standalone InstEventSemaphore
instructions (split_excess_waits).
"""

import numpy as np
import ml_dtypes

import concourse.bass as bass
import concourse.mybir as mybir
import concourse.tile as tile
from concourse.bass_utils import run_bass_kernel_spmd

B, T, C, H = 2, 2048, 1024, 16
D = C // H            # 64 head dim
HC = 4                # heads per core
DC = HC * D           # 256 dcols per core
KT = C // 128         # 8 contraction tiles
NT = T // 128         # 16 t-tiles
TCH = T // 512        # 4 t-chunks of 512
SCALE = 1.0 / np.sqrt(D)

f32 = mybir.dt.float32
f32r = mybir.dt.float32r
bf16 = mybir.dt.bfloat16
BF = ml_dtypes.bfloat16

_CACHE = {}


def _split_excess_waits(nc, max_inline=1):
    """Hoist excess per-instruction waits into standalone event-sem waits."""
    n = 0
    for f in nc.m.functions:
        for bb in f.blocks:
            new_insts = []
            for inst in bb.instructions:
                si = inst.sync_info
                waits = list(si.on_wait) if (si is not None and si.on_wait) else []
                if len(waits) > max_inline:
                    hoist, keep = waits[:-max_inline], waits[-max_inline:]
                    for w in hoist:
                        ev = mybir.InstEventSemaphore(
                            name=nc.get_next_instruction_name(),
                            engine=inst.engine,
                            ins=[],
                            outs=[],
                            sync_info=mybir.SyncInfo(on_wait=[w], on_update=[]),
                        )
                        nc.register_instruction(ev, overwrite=True)
                        new_insts.append(ev)
                        n += 1
                    si.on_wait = keep
                new_insts.append(inst)
            bb.instructions[:] = new_insts
    return n


def _build_program(with_bias=False):
    nc = bass.Bass("TRN2", target_bir_lowering=False, debug=False)

    xt_d = nc.dram_tensor("xt", [C, T], bf16, kind="ExternalInput").ap()
    wq_d = nc.dram_tensor("wq", [C, DC], bf16, kind="ExternalInput").ap()
    wk_d = nc.dram_tensor("wk", [C, DC], bf16, kind="ExternalInput").ap()
    wv_d = nc.dram_tensor("wv", [C, DC], bf16, kind="ExternalInput").ap()
    wp_d = nc.dram_tensor("wp", [DC, C], bf16, kind="ExternalInput").ap()
    if with_bias:
        bq_d = nc.dram_tensor("bq", [DC], f32, kind="ExternalInput").ap()
        bk_d = nc.dram_tensor("bk", [DC], f32, kind="ExternalInput").ap()
        bv_d = nc.dram_tensor("bv", [1, DC], bf16, kind="ExternalInput").ap()
    out_d = nc.dram_tensor("out_t", [C, T], bf16, kind="ExternalOutput").ap()

    with (
        tile.TileContext(nc) as tc,
        nc.allow_low_precision("bf16 compute; tolerance budget is 2e-2"),
    ):
        with (
            tc.tile_pool(name="const", bufs=1) as cpool,
            tc.tile_pool(name="big", bufs=1) as big,
            tc.tile_pool(name="w", bufs=1) as wpool,
            tc.tile_pool(name="pp", bufs=6) as pp,
            tc.tile_pool(name="r1p", bufs=2) as r1p,
            tc.tile_pool(name="rbp", bufs=2) as rbp,
            tc.tile_pool(name="ytp", bufs=2) as ytp,
            tc.tile_pool(name="obp", bufs=3) as obp,
            tc.tile_pool(name="psA", bufs=2, space="PSUM") as psA,
            tc.tile_pool(name="ps_y", bufs=4, space="PSUM") as ps_y,
        ):
            # ---- input DMAs (issue order = DMA execution order) ----
            wq_sb = wpool.tile([128, KT, DC], bf16)
            wk_sb = wpool.tile([128, KT, DC], bf16)
            wv_sb = wpool.tile([128, KT, DC], bf16)
            wp_sb = wpool.tile([128, 2, C], bf16)
            xt_sb = big.tile([128, KT, T], bf16)
            nc.sync.dma_start(
                out=wq_sb[:, 0:4, :],
                in_=wq_d[0:512, :].rearrange("(k p) n -> p k n", k=4))
            for k in range(2):
                nc.sync.dma_start(
                    out=xt_sb[:, k, 0:512],
                    in_=xt_d[bass.ts(k, 128), 0:512])
            nc.sync.dma_start(
                out=wq_sb[:, 4:8, :],
                in_=wq_d[512:1024, :].rearrange("(k p) n -> p k n", k=4))
            for k in range(2, 4):
                nc.sync.dma_start(
                    out=xt_sb[:, k, 0:512],
                    in_=xt_d[bass.ts(k, 128), 0:512])
            nc.sync.dma_start(
                out=wk_sb[:, 0:4, :],
                in_=wk_d[0:512, :].rearrange("(k p) n -> p k n", k=4))
            for k in range(4, 6):
                nc.sync.dma_start(
                    out=xt_sb[:, k, 0:512],
                    in_=xt_d[bass.ts(k, 128), 0:512])
            nc.sync.dma_start(
                out=wk_sb[:, 4:8, :],
                in_=wk_d[512:1024, :].rearrange("(k p) n -> p k n", k=4))
            for k in range(6, KT):
                nc.sync.dma_start(
                    out=xt_sb[:, k, 0:512],
                    in_=xt_d[bass.ts(k, 128), 0:512])
            nc.sync.dma_start(
                out=wv_sb, in_=wv_d.rearrange("(k p) n -> p k n", k=KT))
            nc.sync.dma_start(
                out=xt_sb[:, :, 512:1024],
                in_=xt_d[:, 512:1024].rearrange("(k p) t -> p k t", k=KT))
            nc.sync.dma_start(
                out=wp_sb, in_=wp_d.rearrange("(m p) n -> p m n", m=2))
            for g in (2, 3):
                nc.sync.dma_start(
                    out=xt_sb[:, :, 512 * g:512 * (g + 1)],
                    in_=xt_d[:, 512 * g:512 * (g + 1)].rearrange(
                        "(k p) t -> p k t", k=KT))

            if with_bias:
                bq_sb = cpool.tile([128, 2, 1], f32)
                bk_sb = cpool.tile([128, 2, 1], f32)
                for m in range(2):
                    nc.sync.dma_start(
                        out=bq_sb[:, m, :],
                        in_=bq_d[bass.ds(128 * m, 128)].rearrange(
                            "(p o) -> p o", o=1))
                    nc.sync.dma_start(
                        out=bk_sb[:, m, :],
                        in_=bk_d[bass.ds(128 * m, 128)].rearrange(
                            "(p o) -> p o", o=1))
                bv_sb = cpool.tile([1, DC], bf16)
                nc.sync.dma_start(out=bv_sb, in_=bv_d)
                onesrow = cpool.tile([1, 128], bf16)
                nc.gpsimd.memset(onesrow, 1.0)

            # ---- constants / persistent ----
            ones = cpool.tile([128, D], f32)
            nc.gpsimd.memset(ones, 1.0)
            onesr = ones.bitcast(f32r)
            # mask-as-matmul constants: out[p,f] += sum_k triu[k,p]*diagneg[k,f]
            # = -1e30 where f < p (strictly-future positions of a diag block)
            triu = cpool.tile([128, 128], bf16)
            nc.gpsimd.memset(triu, 1.0)
            nc.gpsimd.affine_select(
                out=triu, in_=triu, compare_op=mybir.AluOpType.is_gt,
                fill=0.0, base=0, pattern=[[1, 128]], channel_multiplier=-1)
            diagneg = cpool.tile([128, 128], bf16)
            nc.gpsimd.memset(diagneg, -1.0e30)
            nc.gpsimd.affine_select(
                out=diagneg, in_=diagneg, compare_op=mybir.AluOpType.is_equal,
                fill=0.0, base=0, pattern=[[1, 128]], channel_multiplier=-1)

            qt = big.tile([128, 2, T], bf16)
            kt = big.tile([128, 2, T], bf16)
            va = big.tile([128, HC, NT, D + 1], bf16)
            yt = big.tile([128, 2, T], bf16)
            for h in range(HC):
                nc.gpsimd.memset(va[:, h, :, D:D + 1], 1.0)

            ident = mybir.ActivationFunctionType.Identity

            def qk_proj(g):
                tsl = bass.ts(g, 512)
                for m in range(2):
                    qkp = psA.tile([128, 1024], f32, name="A")
                    for k in range(KT):
                        nc.tensor.matmul(
                            qkp[:, 0:512],
                            wq_sb[:, k, bass.ts(m, 128)], xt_sb[:, k, tsl],
                            start=(k == 0), stop=(k == KT - 1))
                    for k in range(KT):
                        nc.tensor.matmul(
                            qkp[:, 512:1024],
                            wk_sb[:, k, bass.ts(m, 128)], xt_sb[:, k, tsl],
                            start=(k == 0), stop=(k == KT - 1))
                    if with_bias:
                        nc.scalar.activation(
                            out=qt[:, m, tsl], in_=qkp[:, 0:512], func=ident,
                            bias=bq_sb[:, m, :], scale=1.0)
                        nc.scalar.activation(
                            out=kt[:, m, tsl], in_=qkp[:, 512:1024], func=ident,
                            bias=bk_sb[:, m, :], scale=1.0)
                    else:
                        nc.vector.tensor_copy(
                            out=qt[:, m, tsl], in_=qkp[:, 0:512])
                        nc.vector.tensor_copy(
                            out=kt[:, m, tsl], in_=qkp[:, 512:1024])

            def v_proj(g):
                for half in range(2):
                    it0 = 4 * g + 2 * half
                    vt = psA.tile([128, 1024], f32, name="A")
                    for sub in range(2):
                        it = it0 + sub
                        vp = vt[:, 512 * sub:512 * sub + DC]
                        for k in range(KT):
                            nc.tensor.matmul(
                                vp, xt_sb[:, k, bass.ts(it, 128)],
                                wv_sb[:, k, :], start=(k == 0),
                                stop=(k == KT - 1 and not with_bias))
                        if with_bias:
                            nc.tensor.matmul(
                                vp, onesrow, bv_sb, start=False, stop=True)
                    # one strided copy moves both t-tiles into va
                    nc.vector.tensor_copy(
                        out=va[:, :, it0:it0 + 2, 0:D],
                        in_=vt.rearrange("p (i x) -> p i x", i=2)[:, :, 0:DC]
                            .rearrange("p i (h d) -> p h i d", h=HC))

            def attention(hp, g):
                nj = 4 * g + 4
                yda = ps_y.tile([128, 512], f32, name="yd")
                ydb = ps_y.tile([128, 512], f32, name="yd")
                for j in range(nj):
                    r = j - 4 * g
                    lo = 128 * r if r > 0 else 0
                    tsl = bass.ds(512 * g + lo, 512 - lo)
                    s12 = psA.tile([128, 1024], f32, name="A")
                    diag = r >= 0
                    nc.tensor.matmul(
                        s12[:, lo:512], kt[0:64, hp, bass.ts(j, 128)],
                        qt[0:64, hp, tsl], start=True, stop=not diag)
                    if diag:
                        nc.tensor.matmul(
                            s12[:, lo:lo + 128], triu, diagneg,
                            start=False, stop=True, skip_group_check=True)
                    nc.tensor.matmul(
                        s12[:, 512 + lo:1024], kt[64:128, hp, bass.ts(j, 128)],
                        qt[64:128, hp, tsl], start=True, stop=not diag)
                    if diag:
                        nc.tensor.matmul(
                            s12[:, 512 + lo:512 + lo + 128], triu, diagneg,
                            start=False, stop=True, skip_group_check=True)
                    p12 = pp.tile([128, 1024], bf16, name="p12")
                    sv = s12.rearrange("p (h t) -> p h t", h=2)[:, :, lo:]
                    pv = p12.rearrange("p (h t) -> p h t", h=2)[:, :, lo:]
                    nc.scalar.activation(
                        out=pv, in_=sv,
                        func=mybir.ActivationFunctionType.Exp,
                        scale=float(SCALE))
                    last = j == nj - 1
                    nc.tensor.matmul(
                        yda[0:D + 1, lo:512], va[:, 2 * hp, j, :],
                        p12[:, lo:512], start=(j == 0), stop=last,
                        skip_group_check=True)
                    nc.tensor.matmul(
                        ydb[0:D + 1, lo:512], va[:, 2 * hp + 1, j, :],
                        p12[:, 512 + lo:1024], start=(j == 0), stop=last,
                        skip_group_check=True)
                # normalize both heads: per-head recip + one broadcast pair
                # (returned as a closure so callers can defer the PSUM tile
                # allocation of bc past the next phase's allocations --
                # otherwise that phase's 2nd tile waits on the DVE chain)
                def norm():
                    r1 = r1p.tile([128, 1024], f32r, name="r1")
                    nc.vector.reciprocal(
                        out=r1[64:65, 0:512], in_=yda[64:65, :])
                    nc.vector.reciprocal(
                        out=r1[64:65, 512:1024], in_=ydb[64:65, :])
                    bc = psA.tile([128, 1024], f32, name="A")
                    nc.tensor.matmul(
                        bc[0:64, 0:512], onesr[64:65, :], r1[64:65, 0:512],
                        start=True, stop=True)
                    nc.tensor.matmul(
                        bc[0:64, 512:1024], onesr[64:65, :],
                        r1[64:65, 512:1024], start=True, stop=True)
                    rb = rbp.tile([64, 1024], f32, name="rb")
                    nc.vector.tensor_copy(
                        out=rb[:, 512:1024], in_=bc[0:64, 512:1024])
                    nc.scalar.activation(
                        out=rb[:, 0:512], in_=bc[0:64, 0:512], func=ident,
                        scale=1.0)
                    ytmp = ytp.tile([64, 512], bf16, name="ytmp")
                    nc.vector.tensor_mul(
                        ytmp, ydb[0:64, :], rb[:, 512:1024])
                    nc.gpsimd.tensor_copy(
                        out=yt[64:128, hp, bass.ts(g, 512)], in_=ytmp)
                    nc.vector.tensor_mul(
                        yt[0:64, hp, bass.ts(g, 512)],
                        yda[0:64, :], rb[:, 0:512])
                return norm

            def outproj(g):
                tsl = bass.ts(g, 512)
                for mp in range(4):
                    op = psA.tile([128, 1024], f32, name="A")
                    for sub in range(2):
                        mo = 2 * mp + sub
                        for m in range(2):
                            nc.tensor.matmul(
                                op[:, 512 * sub:512 * (sub + 1)],
                                wp_sb[:, m, bass.ts(mo, 128)], yt[:, m, tsl],
                                start=(m == 0), stop=(m == 1))
                    ob = obp.tile([128, 1024], bf16, name="ob")
                    nc.vector.tensor_copy(
                        out=ob[:, 0:512], in_=op[:, 0:512])
                    nc.scalar.activation(
                        out=ob[:, 512:1024], in_=op[:, 512:1024], func=ident,
                        scale=1.0)
                    nc.sync.dma_start(
                        out=out_d[bass.ds(256 * mp, 256), tsl].rearrange(
                            "(h p) t -> p h t", h=2),
                        in_=ob.rearrange("p (h t) -> p h t", h=2))

            norm_prev = None
            for g in range(TCH):
                qk_proj(g)
                v_proj(g)
                if norm_prev is not None:
                    norm_prev()
                n0 = attention(0, g)
                if g > 0:
                    outproj(g - 1)
                n0()
                norm_prev = attention(1, g)
            norm_prev()
            outproj(TCH - 1)

    _split_excess_waits(nc)
    return nc


def kernel(**inputs) -> np.ndarray:
    query = np.ascontiguousarray(np.asarray(inputs["query"], dtype=np.float32))
    Wq = np.asarray(inputs["Wq"], dtype=np.float32)
    Wk = np.asarray(inputs["Wk"], dtype=np.float32)
    Wv = np.asarray(inputs["Wv"], dtype=np.float32)
    Wp = np.asarray(inputs["Wp"], dtype=np.float32)
    bq = np.asarray(inputs["bq"], dtype=np.float32)
    bk = np.asarray(inputs["bk"], dtype=np.float32)
    bv = np.asarray(inputs["bv"], dtype=np.float32)
    bp = np.asarray(inputs["bp"], dtype=np.float32)
    n_head = int(inputs.get("n_head", H))
    assert n_head == H, f"kernel hardcodes n_head={H}, got {n_head}"
    assert query.shape == (B, T, C)

    with_bias = not (np.all(bq == 0) and np.all(bk == 0) and np.all(bv == 0))
    key = ("nc", with_bias)
    if key not in _CACHE:
        _CACHE[key] = _build_program(with_bias=with_bias)
    nc = _CACHE[key]

    xt_np = [np.ascontiguousarray(query[b].T).astype(BF) for b in range(B)]
    in_maps = []
    for c in range(8):
        b = c // 4
        hg = c % 4
        cols = slice(DC * hg, DC * (hg + 1))
        m = {
            "xt": xt_np[b],
            "wq": np.ascontiguousarray(Wq[:, cols]).astype(BF),
            "wk": np.ascontiguousarray(Wk[:, cols]).astype(BF),
            "wv": np.ascontiguousarray(Wv[:, cols]).astype(BF),
            "wp": np.ascontiguousarray(Wp[cols, :]).astype(BF),
        }
        if with_bias:
            m["bq"] = np.ascontiguousarray(bq[cols])
            m["bk"] = np.ascontiguousarray(bk[cols])
            m["bv"] = np.ascontiguousarray(bv[cols])[None, :].astype(BF)
        in_maps.append(m)

    res = run_bass_kernel_spmd(nc, in_maps, core_ids=list(range(8)))
    _CACHE["last_res"] = res
    _CACHE["last_nc"] = nc

    out = np.empty((B, T, C), np.float32)
    for b in range(B):
        acc = res.results[4 * b]["out_t"].astype(np.float32)
        for c in range(4 * b + 1, 4 * b + 4):
            acc = acc + res.results[c]["out_t"].astype(np.float32)
        out[b] = acc.T + bp
    return out


# revision 13
# speedup vs baseline: 1.0402x; 1.0402x over previous
"""Causal self-attention Bass/TRN2 kernel for nn_CausalSelfAttention.

Shapes (hardcoded): query [2, 2048, 1024], 16 heads, d=64.
Sharding: 8 cores = 2 batches x 4 head-groups (4 heads per core, tensor
parallel on QKV/proj weight columns). Each core computes a partial output
projection out_t = Wp_slice^T @ y^T (shape [1024, 2048] bf16); host sums the
4 partials per batch in f32, transposes, and adds bp.

Host-side prep (free): X transposed to X^T and cast to bf16; weights cast to
bf16. All device compute in bf16 (inputs) with f32 PSUM accumulation.

Per-core pipeline, streamed over 4 query chunks g of 512:
  1. Q^T, K^T chunk = Wq/Wk_slice^T @ X^T chunk (8 k-steps, f32 PSUM,
     ACT Identity copy -> bf16 SBUF, + bias when present).
  2. V tiles = X^T chunk^T-contraction @ Wv_slice (natural [t, d] layout),
     Pool copy into va[s, h, tile, 0:64]; va[..., 64] = 1 (denominator ones).
  3. Attention per head-pair hp: S^T_j = k^T_j ^T @ q^T (2 heads packed in a
     [128,1024] PSUM tile); ACT exp (scale=1/8, no max-subtraction -- scores
     are bounded for this problem) -> bf16 P; causal diag-block zeroing via
     Pool affine_select on P; PV + denominator (ones column of va) into a
     [128,1024] PSUM tile (rows 0:64 y^T, row 64 den).
  4. Normalize: DVE reciprocal of den row, PE K=1 ones-matmul broadcast to 64
     rows, Pool copy PSUM->SBUF, DVE multiply -> yt bf16 (odd head moved to
     partitions 64:128 via SBUF-SBUF DMA).
  5. outproj(g-1) emitted between hp0 and hp1 of chunk g so PE never waits
     on the normalization chain: out_t = Wp_slice^T @ y^T, DVE copy -> bf16,
     DMA out.

This walrus build accepts only ONE sync-wait command per TPB instruction, so
after Tile scheduling we hoist excess waits into standalone InstEventSemaphore
instructions (split_excess_waits).
"""

import numpy as np
import ml_dtypes

import concourse.bass as bass
import concourse.mybir as mybir
import concourse.tile as tile
from concourse.bass_utils import run_bass_kernel_spmd

B, T, C, H = 2, 2048, 1024, 16
D = C // H            # 64 head dim
HC = 4                # heads per core
DC = HC * D           # 256 dcols per core
KT = C // 128         # 8 contraction tiles
NT = T // 128         # 16 t-tiles
TCH = T // 512        # 4 t-chunks of 512
SCALE = 1.0 / np.sqrt(D)

f32 = mybir.dt.float32
f32r = mybir.dt.float32r
bf16 = mybir.dt.bfloat16
BF = ml_dtypes.bfloat16

_CACHE = {}


def _split_excess_waits(nc, max_inline=1):
    """Hoist excess per-instruction waits into standalone event-sem waits."""
    n = 0
    for f in nc.m.functions:
        for bb in f.blocks:
            new_insts = []
            for inst in bb.instructions:
                si = inst.sync_info
                waits = list(si.on_wait) if (si is not None and si.on_wait) else []
                if len(waits) > max_inline:
                    hoist, keep = waits[:-max_inline], waits[-max_inline:]
                    for w in hoist:
                        ev = mybir.InstEventSemaphore(
                            name=nc.get_next_instruction_name(),
                            engine=inst.engine,
                            ins=[],
                            outs=[],
                            sync_info=mybir.SyncInfo(on_wait=[w], on_update=[]),
                        )
                        nc.register_instruction(ev, overwrite=True)
                        new_insts.append(ev)
                        n += 1
                    si.on_wait = keep
                new_insts.append(inst)
            bb.instructions[:] = new_insts
    return n


def _build_program(with_bias=False):
    nc = bass.Bass("TRN2", target_bir_lowering=False, debug=False)

    xt_d = nc.dram_tensor("xt", [C, T], bf16, kind="ExternalInput").ap()
    wq_d = nc.dram_tensor("wq", [C, DC], bf16, kind="ExternalInput").ap()
    wk_d = nc.dram_tensor("wk", [C, DC], bf16, kind="ExternalInput").ap()
    wv_d = nc.dram_tensor("wv", [C, DC], bf16, kind="ExternalInput").ap()
    wp_d = nc.dram_tensor("wp", [DC, C], bf16, kind="ExternalInput").ap()
    if with_bias:
        bq_d = nc.dram_tensor("bq", [DC], f32, kind="ExternalInput").ap()
        bk_d = nc.dram_tensor("bk", [DC], f32, kind="ExternalInput").ap()
        bv_d = nc.dram_tensor("bv", [1, DC], bf16, kind="ExternalInput").ap()
    out_d = nc.dram_tensor("out_t", [C, T], bf16, kind="ExternalOutput").ap()

    with (
        tile.TileContext(nc) as tc,
        nc.allow_low_precision("bf16 compute; tolerance budget is 2e-2"),
    ):
        with (
            tc.tile_pool(name="const", bufs=1) as cpool,
            tc.tile_pool(name="big", bufs=1) as big,
            tc.tile_pool(name="w", bufs=1) as wpool,
            tc.tile_pool(name="pp", bufs=6) as pp,
            tc.tile_pool(name="r1p", bufs=2) as r1p,
            tc.tile_pool(name="rbp", bufs=2) as rbp,
            tc.tile_pool(name="ytp", bufs=2) as ytp,
            tc.tile_pool(name="obp", bufs=3) as obp,
            tc.tile_pool(name="psA", bufs=2, space="PSUM") as psA,
            tc.tile_pool(name="ps_y", bufs=4, space="PSUM") as ps_y,
        ):
            # ---- input DMAs (issue order = DMA execution order) ----
            wq_sb = wpool.tile([128, KT, DC], bf16)
            wk_sb = wpool.tile([128, KT, DC], bf16)
            wv_sb = wpool.tile([128, KT, DC], bf16)
            wp_sb = wpool.tile([128, 2, C], bf16)
            xt_sb = big.tile([128, KT, T], bf16)
            nc.sync.dma_start(
                out=wq_sb[:, 0:4, :],
                in_=wq_d[0:512, :].rearrange("(k p) n -> p k n", k=4))
            for k in range(2):
                nc.sync.dma_start(
                    out=xt_sb[:, k, 0:512],
                    in_=xt_d[bass.ts(k, 128), 0:512])
            nc.sync.dma_start(
                out=wq_sb[:, 4:8, :],
                in_=wq_d[512:1024, :].rearrange("(k p) n -> p k n", k=4))
            for k in range(2, 4):
                nc.sync.dma_start(
                    out=xt_sb[:, k, 0:512],
                    in_=xt_d[bass.ts(k, 128), 0:512])
            nc.sync.dma_start(
                out=wk_sb[:, 0:4, :],
                in_=wk_d[0:512, :].rearrange("(k p) n -> p k n", k=4))
            for k in range(4, 6):
                nc.sync.dma_start(
                    out=xt_sb[:, k, 0:512],
                    in_=xt_d[bass.ts(k, 128), 0:512])
            nc.sync.dma_start(
                out=wk_sb[:, 4:8, :],
                in_=wk_d[512:1024, :].rearrange("(k p) n -> p k n", k=4))
            for k in range(6, KT):
                nc.sync.dma_start(
                    out=xt_sb[:, k, 0:512],
                    in_=xt_d[bass.ts(k, 128), 0:512])
            nc.sync.dma_start(
                out=wv_sb, in_=wv_d.rearrange("(k p) n -> p k n", k=KT))
            nc.sync.dma_start(
                out=xt_sb[:, :, 512:1024],
                in_=xt_d[:, 512:1024].rearrange("(k p) t -> p k t", k=KT))
            nc.sync.dma_start(
                out=wp_sb, in_=wp_d.rearrange("(m p) n -> p m n", m=2))
            for g in (2, 3):
                nc.sync.dma_start(
                    out=xt_sb[:, :, 512 * g:512 * (g + 1)],
                    in_=xt_d[:, 512 * g:512 * (g + 1)].rearrange(
                        "(k p) t -> p k t", k=KT))

            if with_bias:
                bq_sb = cpool.tile([128, 2, 1], f32)
                bk_sb = cpool.tile([128, 2, 1], f32)
                for m in range(2):
                    nc.sync.dma_start(
                        out=bq_sb[:, m, :],
                        in_=bq_d[bass.ds(128 * m, 128)].rearrange(
                            "(p o) -> p o", o=1))
                    nc.sync.dma_start(
                        out=bk_sb[:, m, :],
                        in_=bk_d[bass.ds(128 * m, 128)].rearrange(
                            "(p o) -> p o", o=1))
                bv_sb = cpool.tile([1, DC], bf16)
                nc.sync.dma_start(out=bv_sb, in_=bv_d)
                onesrow = cpool.tile([1, 128], bf16)
                nc.gpsimd.memset(onesrow, 1.0)

            # ---- constants / persistent ----
            ones = cpool.tile([128, D], f32)
            nc.gpsimd.memset(ones, 1.0)
            onesr = ones.bitcast(f32r)
            # mask-as-matmul constants: out[p,f] += sum_k triu[k,p]*diagneg[k,f]
            # = -1e30 where f < p (strictly-future positions of a diag block)
            triu = cpool.tile([128, 128], bf16)
            nc.gpsimd.memset(triu, 1.0)
            nc.gpsimd.affine_select(
                out=triu, in_=triu, compare_op=mybir.AluOpType.is_gt,
                fill=0.0, base=0, pattern=[[1, 128]], channel_multiplier=-1)
            diagneg = cpool.tile([128, 128], bf16)
            nc.gpsimd.memset(diagneg, -1.0e30)
            nc.gpsimd.affine_select(
                out=diagneg, in_=diagneg, compare_op=mybir.AluOpType.is_equal,
                fill=0.0, base=0, pattern=[[1, 128]], channel_multiplier=-1)

            qt = big.tile([128, 2, T], bf16)
            kt = big.tile([128, 2, T], bf16)
            va = big.tile([128, HC, NT, D + 1], bf16)
            yt = big.tile([128, 2, T], bf16)
            for h in range(HC):
                nc.gpsimd.memset(va[:, h, :, D:D + 1], 1.0)

            ident = mybir.ActivationFunctionType.Identity

            def qk_proj(g):
                tsl = bass.ts(g, 512)
                for m in range(2):
                    qkp = psA.tile([128, 1024], f32, name="A")
                    for k in range(KT):
                        nc.tensor.matmul(
                            qkp[:, 0:512],
                            wq_sb[:, k, bass.ts(m, 128)], xt_sb[:, k, tsl],
                            start=(k == 0), stop=(k == KT - 1))
                    for k in range(KT):
                        nc.tensor.matmul(
                            qkp[:, 512:1024],
                            wk_sb[:, k, bass.ts(m, 128)], xt_sb[:, k, tsl],
                            start=(k == 0), stop=(k == KT - 1))
                    if with_bias:
                        nc.scalar.activation(
                            out=qt[:, m, tsl], in_=qkp[:, 0:512], func=ident,
                            bias=bq_sb[:, m, :], scale=1.0)
                        nc.scalar.activation(
                            out=kt[:, m, tsl], in_=qkp[:, 512:1024], func=ident,
                            bias=bk_sb[:, m, :], scale=1.0)
                    else:
                        nc.vector.tensor_copy(
                            out=qt[:, m, tsl], in_=qkp[:, 0:512])
                        nc.vector.tensor_copy(
                            out=kt[:, m, tsl], in_=qkp[:, 512:1024])

            def v_proj(g):
                for half in range(2):
                    it0 = 4 * g + 2 * half
                    vt = psA.tile([128, 1024], f32, name="A")
                    for sub in range(2):
                        it = it0 + sub
                        vp = vt[:, 512 * sub:512 * sub + DC]
                        for k in range(KT):
                            nc.tensor.matmul(
                                vp, xt_sb[:, k, bass.ts(it, 128)],
                                wv_sb[:, k, :], start=(k == 0),
                                stop=(k == KT - 1 and not with_bias))
                        if with_bias:
                            nc.tensor.matmul(
                                vp, onesrow, bv_sb, start=False, stop=True)
                    # one strided copy moves both t-tiles into va
                    nc.vector.tensor_copy(
                        out=va[:, :, it0:it0 + 2, 0:D],
                        in_=vt.rearrange("p (i x) -> p i x", i=2)[:, :, 0:DC]
                            .rearrange("p i (h d) -> p h i d", h=HC))

            def attention(hp, g, fillers=()):
                fillers = list(fillers)
                nj = 4 * g + 4
                yda = ps_y.tile([128, 512], f32, name="yd")
                ydb = ps_y.tile([128, 512], f32, name="yd")
                for j in range(nj):
                    if j >= 2 and fillers:
                        fillers.pop(0)()
                    r = j - 4 * g
                    lo = 128 * r if r > 0 else 0
                    tsl = bass.ds(512 * g + lo, 512 - lo)
                    s12 = psA.tile([128, 1024], f32, name="A")
                    diag = r >= 0
                    nc.tensor.matmul(
                        s12[:, lo:512], kt[0:64, hp, bass.ts(j, 128)],
                        qt[0:64, hp, tsl], start=True, stop=not diag)
                    if diag:
                        nc.tensor.matmul(
                            s12[:, lo:lo + 128], triu, diagneg,
                            start=False, stop=True, skip_group_check=True)
                    nc.tensor.matmul(
                        s12[:, 512 + lo:1024], kt[64:128, hp, bass.ts(j, 128)],
                        qt[64:128, hp, tsl], start=True, stop=not diag)
                    if diag:
                        nc.tensor.matmul(
                            s12[:, 512 + lo:512 + lo + 128], triu, diagneg,
                            start=False, stop=True, skip_group_check=True)
                    _ = None
                    p12 = pp.tile([128, 1024], bf16, name="p12")
                    sv = s12.rearrange("p (h t) -> p h t", h=2)[:, :, lo:]
                    pv = p12.rearrange("p (h t) -> p h t", h=2)[:, :, lo:]
                    nc.scalar.activation(
                        out=pv, in_=sv,
                        func=mybir.ActivationFunctionType.Exp,
                        scale=float(SCALE))
                    last = j == nj - 1
                    nc.tensor.matmul(
                        yda[0:D + 1, lo:512], va[:, 2 * hp, j, :],
                        p12[:, lo:512], start=(j == 0), stop=last,
                        skip_group_check=True)
                    nc.tensor.matmul(
                        ydb[0:D + 1, lo:512], va[:, 2 * hp + 1, j, :],
                        p12[:, 512 + lo:1024], start=(j == 0), stop=last,
                        skip_group_check=True)
                for f in fillers:
                    f()
                # normalize both heads: per-head recip + one broadcast pair
                # (returned as a closure so callers can defer the PSUM tile
                # allocation of bc past the next phase's allocations --
                # otherwise that phase's 2nd tile waits on the DVE chain)
                def norm():
                    r1 = r1p.tile([128, 1024], f32r, name="r1")
                    nc.vector.reciprocal(
                        out=r1[64:65, 0:512], in_=yda[64:65, :])
                    nc.vector.reciprocal(
                        out=r1[64:65, 512:1024], in_=ydb[64:65, :])
                    bc = psA.tile([128, 1024], f32, name="A")
                    nc.tensor.matmul(
                        bc[0:64, 0:512], onesr[64:65, :], r1[64:65, 0:512],
                        start=True, stop=True)
                    nc.tensor.matmul(
                        bc[0:64, 512:1024], onesr[64:65, :],
                        r1[64:65, 512:1024], start=True, stop=True)
                    rb = rbp.tile([64, 1024], f32, name="rb")
                    nc.vector.tensor_copy(out=rb, in_=bc[0:64, :])
                    ytmp = ytp.tile([64, 512], bf16, name="ytmp")
                    nc.vector.tensor_mul(
                        ytmp, ydb[0:64, :], rb[:, 512:1024])
                    nc.gpsimd.tensor_copy(
                        out=yt[64:128, hp, bass.ts(g, 512)], in_=ytmp)
                    nc.vector.tensor_mul(
                        yt[0:64, hp, bass.ts(g, 512)],
                        yda[0:64, :], rb[:, 0:512])
                return norm

            def outproj_tile(g, mp):
                tsl = bass.ts(g, 512)
                op = psA.tile([128, 1024], f32, name="A")
                for sub in range(2):
                    mo = 2 * mp + sub
                    for m in range(2):
                        nc.tensor.matmul(
                            op[:, 512 * sub:512 * (sub + 1)],
                            wp_sb[:, m, bass.ts(mo, 128)], yt[:, m, tsl],
                            start=(m == 0), stop=(m == 1))
                ob = obp.tile([128, 1024], bf16, name="ob")
                nc.vector.tensor_copy(out=ob, in_=op)
                nc.sync.dma_start(
                    out=out_d[bass.ds(256 * mp, 256), tsl].rearrange(
                        "(h p) t -> p h t", h=2),
                    in_=ob.rearrange("p (h t) -> p h t", h=2))

            def outproj(g):
                for mp in range(4):
                    outproj_tile(g, mp)

            import functools
            norm_prev = None
            for g in range(TCH):
                qk_proj(g)
                v_proj(g)
                f0 = [norm_prev] if norm_prev is not None else []
                n0 = attention(0, g, fillers=f0)
                f1 = [n0]
                if g > 0:
                    f1 += [functools.partial(outproj_tile, g - 1, mp)
                           for mp in range(4)]
                norm_prev = attention(1, g, fillers=f1)
            norm_prev()
            outproj(TCH - 1)

    _split_excess_waits(nc)
    return nc


def kernel(**inputs) -> np.ndarray:
    query = np.ascontiguousarray(np.asarray(inputs["query"], dtype=np.float32))
    Wq = np.asarray(inputs["Wq"], dtype=np.float32)
    Wk = np.asarray(inputs["Wk"], dtype=np.float32)
    Wv = np.asarray(inputs["Wv"], dtype=np.float32)
    Wp = np.asarray(inputs["Wp"], dtype=np.float32)
    bq = np.asarray(inputs["bq"], dtype=np.float32)
    bk = np.asarray(inputs["bk"], dtype=np.float32)
    bv = np.asarray(inputs["bv"], dtype=np.float32)
    bp = np.asarray(inputs["bp"], dtype=np.float32)
    n_head = int(inputs.get("n_head", H))
    assert n_head == H, f"kernel hardcodes n_head={H}, got {n_head}"
    assert query.shape == (B, T, C)

    with_bias = not (np.all(bq == 0) and np.all(bk == 0) and np.all(bv == 0))
    key = ("nc", with_bias)
    if key not in _CACHE:
        _CACHE[key] = _build_program(with_bias=with_bias)
    nc = _CACHE[key]

    xt_np = [np.ascontiguousarray(query[b].T).astype(BF) for b in range(B)]
    in_maps = []
    for c in range(8):
        b = c // 4
        hg = c % 4
        cols = slice(DC * hg, DC * (hg + 1))
        m = {
            "xt": xt_np[b],
            "wq": np.ascontiguousarray(Wq[:, cols]).astype(BF),
            "wk": np.ascontiguousarray(Wk[:, cols]).astype(BF),
            "wv": np.ascontiguousarray(Wv[:, cols]).astype(BF),
            "wp": np.ascontiguousarray(Wp[cols, :]).astype(BF),
        }
        if with_bias:
            m["bq"] = np.ascontiguousarray(bq[cols])
            m["bk"] = np.ascontiguousarray(bk[cols])
            m["bv"] = np.ascontiguousarray(bv[cols])[None, :].astype(BF)
        in_maps.append(m)

    res = run_bass_kernel_spmd(nc, in_maps, core_ids=list(range(8)))
    _CACHE["last_res"] = res
    _CACHE["last_nc"] = nc

    out = np.empty((B, T, C), np.float32)
    for b in range(B):
        acc = res.results[4 * b]["out_t"].astype(np.float32)
        for c in range(4 * b + 1, 4 * b + 4):
            acc = acc + res.results[c]["out_t"].astype(np.float32)
        out[b] = acc.T + bp
    return out
